# revision 12
# baseline (speedup 1.0000x reference)
"""Trainium2 Bass kernel for nn_Correction (nms_detection).

Strategy: data-parallel over batch (1 batch per NeuronCore, 8 cores).
  NEFF1 (device): desc = relu(conv3x3(feats, w_pa) + b_pa)  -- fp32 matmuls
                  (the precision-critical, FLOP-dominant stage)
  host:           scores = sigmoid(1x1conv(desc)); simple_nms; top-k;
                  gather+normalize kd; cross-batch attention (gnn); proj;
                  mind -> theta -> affine grid -> gather offsets + bilinear
                  weights  (all <0.1% of total FLOPs)
  NEFF2 (device): out = bilinear grid_sample of feats via indirect-DMA
                  row-pair gathers + per-partition weighted combine
"""

import functools
import numpy as np

import concourse.bacc as bacc
import concourse.bass as bass
import concourse.mybir as mybir
import concourse.tile as tile
from concourse.bass import IndirectOffsetOnAxis
from concourse.bass_utils import run_bass_kernel_spmd

B, C, H, W = 8, 256, 128, 384
CH = 128            # C // 2, desc channels
HW = H * W          # 49152
K = 1024            # MAX_KPTS
NMS_R = 4
NCORES = 8
F32 = mybir.dt.float32
I32 = mybir.dt.int32

# test.py can flip these to profile
TRACE = False
LAST_RESULTS = {}

# ----------------------------------------------------------------------------
# NEFF 1: fp32 3x3 conv + bias + relu.   feats [256,128,384] -> desc [128,128,384]
# ----------------------------------------------------------------------------

CONV_BF16 = True   # bf16 hi/lo split (3 passes) instead of fp32 (4 cyc/row)
BF16 = mybir.dt.bfloat16


@functools.lru_cache(maxsize=1)
def _build_conv():
    nc = bacc.Bacc("TRN2", target_bir_lowering=False, debug=False, num_devices=NCORES)
    if CONV_BF16:
        fh_d = nc.dram_tensor("feats_hi", [C, H, W], BF16, kind="ExternalInput")
        fl_d = nc.dram_tensor("feats_lo", [C, H, W], BF16, kind="ExternalInput")
        wh_d = nc.dram_tensor("w_hi", [128, 18 * 128], BF16, kind="ExternalInput")
        wl_d = nc.dram_tensor("w_lo", [128, 18 * 128], BF16, kind="ExternalInput")
        feat_aps = [fh_d.ap(), fl_d.ap()]
    else:
        feats_d = nc.dram_tensor("feats", [C, H, W], F32, kind="ExternalInput")
        w_d = nc.dram_tensor("w_all", [128, 18 * 128], F32, kind="ExternalInput")
        feat_aps = [feats_d.ap()]
    b_d = nc.dram_tensor("bias", [128, 1], F32, kind="ExternalInput")
    desc_d = nc.dram_tensor("desc", [CH, H, W], F32, kind="ExternalOutput")
    desc_ap = desc_d.ap()
    rdt = BF16 if CONV_BF16 else F32
    nparts = len(feat_aps)  # hi/lo parts of the input

    with tile.TileContext(nc) as tc:
        with (
            tc.tile_pool(name="const", bufs=1) as constp,
            tc.tile_pool(name="rows", bufs=10) as rowp,
            tc.tile_pool(name="out", bufs=3) as outp,
            tc.tile_pool(name="ps", bufs=2, space="PSUM") as psp,
        ):
            if CONV_BF16:
                w_hi = constp.tile([128, 18 * 128], BF16)
                nc.sync.dma_start(w_hi[:], wh_d.ap())
                w_lo = constp.tile([128, 18 * 128], BF16)
                nc.sync.dma_start(w_lo[:], wl_d.ap())
            else:
                w_all = constp.tile([128, 18 * 128], F32)
                nc.sync.dma_start(w_all[:], w_d.ap())
            bias_t = constp.tile([128, 1], F32)
            nc.sync.dma_start(bias_t[:], b_d.ap())
            zrow = [constp.tile([128, W + 2], rdt, tag=f"z{g}{v}", name=f"zrow{g}{v}")
                    for g in range(2) for v in range(nparts)]
            for z in zrow:
                nc.gpsimd.memset(z[:], 0.0)

            def load_row(h):
                # [(g0 hi, g0 lo), (g1 hi, g1 lo)] for image row h (zeros if OOB)
                if h < 0 or h >= H:
                    return [zrow[0:nparts], zrow[nparts:2 * nparts]]
                out = []
                for g in range(2):
                    tv = []
                    for v in range(nparts):
                        t = rowp.tile([128, W + 2], rdt, tag=f"row{g}{v}",
                                      name=f"row{g}{v}")
                        nc.gpsimd.memset(t[:, 0:1], 0.0)
                        nc.gpsimd.memset(t[:, W + 1:W + 2], 0.0)
                        nc.sync.dma_start(t[:, 1:W + 1],
                                          feat_aps[v][g * 128:(g + 1) * 128, h, :])
                        tv.append(t)
                    out.append(tv)
                return out

            window = {}  # h -> [[g0 parts], [g1 parts]]
            for h in range(H):
                for hh in (h - 1, h, h + 1):
                    if hh not in window:
                        window[hh] = load_row(hh)
                ps = psp.tile([128, W], F32)
                nmm = 18 * (3 if CONV_BF16 else 1)
                k = 0
                mm = 0
                for ky in range(3):
                    for kx in range(3):
                        rt = window[h + ky - 1]
                        for g in range(2):
                            ws = slice(k * 128, (k + 1) * 128)
                            if CONV_BF16:
                                # w_hi*x_hi + w_hi*x_lo + w_lo*x_hi
                                for wt, xv in ((w_hi, 0), (w_hi, 1), (w_lo, 0)):
                                    nc.tensor.matmul(
                                        ps[:], wt[:, ws], rt[g][xv][:, kx:kx + W],
                                        start=(mm == 0), stop=(mm == nmm - 1))
                                    mm += 1
                            else:
                                nc.tensor.matmul(
                                    ps[:], w_all[:, ws], rt[g][0][:, kx:kx + W],
                                    start=(mm == 0), stop=(mm == nmm - 1))
                                mm += 1
                            k += 1
                ot = outp.tile([128, W], F32)
                nc.scalar.activation(ot[:], ps[:], mybir.ActivationFunctionType.Relu,
                                     bias=bias_t[:, 0:1], scale=1.0)
                nc.sync.dma_start(desc_ap[:, h, :], ot[:])
                # drop the row leaving the window
                window.pop(h - 1, None)
    nc.compile()
    return nc


# ----------------------------------------------------------------------------
# NEFF 2: grid_sample over valid pixels only, redistributed across all cores.
#   Each core gets 4 DRAM "band" tensors (65 image rows of some batch, fp16).
#   Pixels are packed into superchunks of 512; chunk j gathers from band
#   j // (nchunk//4).  Per pixel: 2 elements of 2 px (1024B) on rows y0/y1,
#   4 weights.  prepare_only gathers + trigger_dma keep GPSIMD free.
# ----------------------------------------------------------------------------

NSUB = 4            # 4 x 128 pixels per superchunk
BROWS = 65          # rows per band tensor
BANDPX = BROWS * W  # 24960 pixel-rows per band
GE2 = 2 * C         # gather element: 2 pixel-rows (1024B fp16)
GSTEP2 = C          # element stride: 1 pixel-row -> idx = px index, fits int16
NQ2 = BANDPX - 1
FP16 = mybir.dt.float16
PREP_GATHER = False   # prepare_only + trigger_dma (frees GPSIMD) vs blocking


@functools.lru_cache(maxsize=4)
def _build_sample2(nchunk, prep=PREP_GATHER):
    nc = bacc.Bacc("TRN2", target_bir_lowering=False, debug=False, num_devices=NCORES)
    band_d = [nc.dram_tensor(f"band{v}", [BANDPX, C], FP16, kind="ExternalInput")
              for v in range(4)]
    idx_d = nc.dram_tensor("idx", [128, nchunk * 64], mybir.dt.int16,
                           kind="ExternalInput")
    wts_d = nc.dram_tensor("wts", [128, nchunk * 16], FP16, kind="ExternalInput")
    out_d = nc.dram_tensor("out_t", [nchunk * 512, C], F32, kind="ExternalOutput")
    band_views = [bass.AP(d.ap().tensor, 0, [[GSTEP2, NQ2], [1, GE2]])
                  for d in band_d]
    out_ap = out_d.ap()
    cpb = nchunk // 4   # chunks per band

    with tile.TileContext(nc) as tc:
        with (
            tc.tile_pool(name="const", bufs=1) as constp,
            tc.tile_pool(name="gat", bufs=6) as gatp,
            tc.tile_pool(name="prod", bufs=12) as prodp,
            tc.tile_pool(name="out", bufs=4) as outp,
            tc.tile_pool(name="ps", bufs=6, space="PSUM") as psp,
        ):
            idx_t = constp.tile([128, nchunk * 64], mybir.dt.int16)
            nc.sync.dma_start(idx_t[:], idx_d.ap())
            wts_t = constp.tile([128, nchunk * 16], FP16)
            nc.sync.dma_start(wts_t[:], wts_d.ap())
            ones_t = constp.tile([128, 128], FP16)
            nc.gpsimd.memset(ones_t[:], 1.0)
            ident = constp.tile([128, 128], FP16)
            nc.gpsimd.affine_select(ident[:], ones_t[:], pattern=[[1, 128]],
                                    compare_op=mybir.AluOpType.is_equal, fill=0.0,
                                    base=0, channel_multiplier=-1)
            dma_sem = nc.alloc_semaphore("gat_dma")

            for j in range(nchunk):
                g = gatp.tile([128, 2 * NSUB * GE2], FP16)
                if prep:
                    nc.gpsimd.dma_gather(
                        g[:].rearrange("p (i e) -> p i e", e=GE2),
                        band_views[j // cpb], idx_t[:, j * 64:(j + 1) * 64],
                        num_idxs=2 * NSUB * 128, num_idxs_reg=2 * NSUB * 128,
                        elem_size=GE2, elem_step=GSTEP2,
                        prepare_only=True, sem=dma_sem)
                    nc.gpsimd.trigger_dma(count=None)
                else:
                    nc.gpsimd.dma_gather(
                        g[:].rearrange("p (i e) -> p i e", e=GE2),
                        band_views[j // cpb], idx_t[:, j * 64:(j + 1) * 64],
                        num_idxs=2 * NSUB * 128, num_idxs_reg=2 * NSUB * 128,
                        elem_size=GE2, elem_step=GSTEP2)
                ot = outp.tile([128, NSUB * C], F32)
                for s in range(NSUB):
                    ps = psp.tile([128, C], F32)
                    # one DVE mul for all 4 taps: [128, (4,256)] * w[128,(4,1->256)]
                    pr = prodp.tile([128, 4 * C], FP16, tag="pr")
                    src = g[:, 2 * s * GE2:2 * s * GE2 + 4 * C]
                    w4 = wts_t[:, j * 16 + s * 4:j * 16 + s * 4 + 4]
                    w_b = bass.AP(w4.tensor, w4.offset, list(w4.ap) + [[0, C]])
                    nc.vector.tensor_tensor(
                        pr[:].rearrange("p (k c) -> p k c", c=C),
                        src.rearrange("p (k c) -> p k c", c=C),
                        w_b, mybir.AluOpType.mult)
                    for k2 in range(4):
                        nc.tensor.matmul(ps[:], ident[:],
                                         pr[:, k2 * C:(k2 + 1) * C],
                                         start=(k2 == 0), stop=(k2 == 3))
                    nc.scalar.activation(ot[:, s * C:(s + 1) * C], ps[:],
                                         mybir.ActivationFunctionType.Copy, scale=1.0)
                dst = out_ap[j * 512:(j + 1) * 512, :].rearrange("(s p) c -> p s c", p=128)
                nc.sync.dma_start(dst, ot[:].rearrange("p (s c) -> p s c", s=NSUB))
    nc.compile()
    return nc


# ----------------------------------------------------------------------------
# Host-side middle stages (tiny compute)
# ----------------------------------------------------------------------------

def _max_pool(x, r):
    b, h, w = x.shape
    k = 2 * r + 1
    xp = np.pad(x, ((0, 0), (r, r), (r, r)), constant_values=-np.inf)
    out = np.full((b, h, w), -np.inf, dtype=x.dtype)
    for dy in range(k):
        for dx in range(k):
            np.maximum(out, xp[:, dy:dy + h, dx:dx + w], out=out)
    return out


def _simple_nms(scores, r):
    zeros = np.zeros_like(scores)
    max_mask = scores == _max_pool(scores, r)
    for _ in range(2):
        supp_mask = _max_pool(max_mask.astype(scores.dtype), r) > 0
        supp_scores = np.where(supp_mask, zeros, scores)
        new_max_mask = supp_scores == _max_pool(supp_scores, r)
        max_mask = max_mask | (new_max_mask & ~supp_mask)
    return np.where(max_mask, scores, zeros)


def _host_middle(desc, feats, w_pb, b_pb, w_proj, b_proj):
    """desc [B, CH, HW] f32 (device conv output) -> theta [B, 2, 3] f32."""
    pre = np.einsum('bcp,c->bp', desc, w_pb[:, :, 0, 0][0], optimize=True) + b_pb[0]
    scores = 1.0 / (1.0 + np.exp(-pre.astype(np.float32)))
    scores = _simple_nms(scores.reshape(B, H, W), NMS_R).reshape(B, -1)
    idx = np.argsort(-scores, axis=1, kind='stable')[:, :K]          # [B, K]
    kd = np.take_along_axis(desc, idx[:, None, :], axis=2)           # [B, CH, K]
    norm = np.sqrt(np.sum(kd * kd, axis=1, keepdims=True))
    kd = kd / np.maximum(norm, 1e-12)
    # gnn: per-keypoint attention across the batch dim
    q = np.transpose(kd, (2, 0, 1)).astype(np.float32)               # [K, B, CH]
    sc = np.einsum('lnc,lmc->lnm', q, q, optimize=True) / np.float32(np.sqrt(CH))
    sc = sc - sc.max(axis=-1, keepdims=True)
    e = np.exp(sc)
    prob = e / e.sum(-1, keepdims=True)
    msg = np.einsum('lnm,lmc->lnc', prob, q, optimize=True)
    kd2 = kd + (kd + np.transpose(msg, (1, 2, 0)))
    proj = np.einsum('bcl,oc->bol', kd2, w_proj[:, :, 0], optimize=True) \
        + b_proj[None, :, None]
    proj = proj - proj[0:1]
    mind = proj.min(axis=2).astype(np.float32)                       # [B, 3]
    c, s = np.cos(mind[:, 2]), np.sin(mind[:, 2])
    theta = np.stack([np.stack([c, -s, mind[:, 0]], -1),
                      np.stack([s, c, mind[:, 1]], -1)], axis=1).astype(np.float32)
    return theta


def _grid_tables(theta):
    """theta [B,2,3] -> off [B, HW, 2] int32 row starts, wts [B, HW, 4] f32."""
    xs = ((np.arange(W, dtype=np.float32) * 2 + 1) / W - 1)
    ys = ((np.arange(H, dtype=np.float32) * 2 + 1) / H - 1)
    gxm, gym = np.meshgrid(xs, ys)                                   # [H, W]
    offs, wtss = [], []
    for b in range(B):
        t = theta[b]
        grid_x = gxm * t[0, 0] + gym * t[0, 1] + t[0, 2]
        grid_y = gxm * t[1, 0] + gym * t[1, 1] + t[1, 2]
        gx = (grid_x + 1) * W / 2 - 0.5
        gy = (grid_y + 1) * H / 2 - 0.5
        x0 = np.floor(gx)
        y0 = np.floor(gy)
        wx1 = (gx - x0).astype(np.float32); wx0 = 1.0 - wx1
        wy1 = (gy - y0).astype(np.float32); wy0 = 1.0 - wy1

        def v(xi, yi):
            return ((xi >= 0) & (xi < W) & (yi >= 0) & (yi < H)).astype(np.float32)
        w00 = wx0 * wy0 * v(x0, y0)
        w01 = wx1 * wy0 * v(x0 + 1, y0)
        w10 = wx0 * wy1 * v(x0, y0 + 1)
        w11 = wx1 * wy1 * v(x0 + 1, y0 + 1)
        x0i = x0.astype(np.int64)
        xs_ = np.clip(x0i, 0, W - 2)
        wa0 = w00 * (xs_ == x0i) + w01 * (xs_ == x0i + 1)
        wb0 = w00 * (xs_ + 1 == x0i) + w01 * (xs_ + 1 == x0i + 1)
        wa1 = w10 * (xs_ == x0i) + w11 * (xs_ == x0i + 1)
        wb1 = w10 * (xs_ + 1 == x0i) + w11 * (xs_ + 1 == x0i + 1)
        y0i = y0.astype(np.int64)
        y0c = np.clip(y0i, 0, H - 1)
        y1c = np.clip(y0i + 1, 0, H - 1)
        off0 = (y0c * W + xs_).astype(np.int32)
        off1 = (y1c * W + xs_).astype(np.int32)
        offs.append(np.stack([off0.reshape(-1), off1.reshape(-1)], -1))
        wtss.append(np.stack([wa0.reshape(-1), wb0.reshape(-1),
                              wa1.reshape(-1), wb1.reshape(-1)], -1).astype(np.float32))
    return np.stack(offs), np.stack(wtss)


# ----------------------------------------------------------------------------
# kernel()
# ----------------------------------------------------------------------------

def kernel(feats, w_pa, b_pa, w_pb, b_pb, w_proj, b_proj):
    import ml_dtypes
    feats = np.ascontiguousarray(feats, dtype=np.float32)
    # weights for the conv matmuls: block k=((ky*3+kx)*2+g): lhsT[ci, co]
    wr = w_pa.reshape(128, 2, 128, 3, 3).transpose(2, 3, 4, 1, 0)   # ci,ky,kx,g,co
    w_all = np.ascontiguousarray(wr.reshape(128, 18 * 128), dtype=np.float32)
    bias = np.ascontiguousarray(b_pa.reshape(128, 1), dtype=np.float32)

    nc1 = _build_conv()
    if CONV_BF16:
        f_hi = feats.astype(ml_dtypes.bfloat16)
        f_lo = (feats - f_hi.astype(np.float32)).astype(ml_dtypes.bfloat16)
        w_hi = w_all.astype(ml_dtypes.bfloat16)
        w_lo = (w_all - w_hi.astype(np.float32)).astype(ml_dtypes.bfloat16)
        in_maps = [{"feats_hi": f_hi[b], "feats_lo": f_lo[b],
                    "w_hi": w_hi, "w_lo": w_lo, "bias": bias} for b in range(B)]
    else:
        in_maps = [{"feats": feats[b], "w_all": w_all, "bias": bias} for b in range(B)]
    r1 = run_bass_kernel_spmd(nc1, in_maps, core_ids=list(range(NCORES)), trace=TRACE)
    LAST_RESULTS["conv"] = r1
    desc = np.stack([r1.results[b]["desc"] for b in range(B)])       # [B, CH, H, W]

    theta = _host_middle(desc.reshape(B, CH, HW), feats, w_pb, b_pb, w_proj, b_proj)
    off, wts = _grid_tables(theta)                                   # [B,HW,2],[B,HW,4]

    # batches whose theta is exactly identity sample the image exactly -> copy
    ident_b = [b for b in range(B)
               if np.abs(theta[b] - np.array([[1, 0, 0], [0, 1, 0]],
                                             dtype=np.float32)).max() < 1e-6]
    work_b = [b for b in range(B) if b not in ident_b]

    # valid pixels: at least one of the 4 bilinear weights nonzero
    wsum = wts.sum(axis=2)                                           # [B, HW]
    slots = None
    for nchunk in (48, 56, 64, 80, 96, 112, 128, 160, 192):
        cap = (nchunk // 4) * 512
        cand = []
        feasible = True
        for b in work_b:
            pxs = np.nonzero(wsum[b] > 0)[0].astype(np.int64)
            if pxs.size == 0:
                continue
            y0c = off[b, pxs, 0] // W                                # source row of y0 tap
            order = np.argsort(y0c, kind="stable")
            pxs, y0c = pxs[order], y0c[order]
            i = 0
            while i < pxs.size:
                ylo = int(y0c[i])
                j = min(i + cap, int(np.searchsorted(y0c, ylo + 64, "left")))
                cand.append((b, ylo, pxs[i:j]))
                i = j
            if len(cand) > 32:
                feasible = False
                break
        if feasible and len(cand) <= 32:
            slots = cand
            break
    assert slots is not None, "could not pack valid pixels into 32 slots"

    nc2 = _build_sample2(nchunk, PREP_GATHER)
    cpb = nchunk // 4
    ft = {b: feats[b].reshape(C, HW).T.astype(np.float16) for b in work_b}

    zero_band = np.zeros((BANDPX, C), dtype=np.float16)
    in_maps2 = []
    meta = []                                                        # per core: (b, px) rows
    for core in range(NCORES):
        m = {"idx": np.zeros((128, nchunk * 64), dtype=np.int16),
             "wts": np.zeros((128, nchunk * 16), dtype=np.float16)}
        rows_b = np.full(nchunk * 512, -1, dtype=np.int64)
        rows_px = np.zeros(nchunk * 512, dtype=np.int64)
        for pos in range(4):
            k = core * 4 + pos
            if k >= len(slots):
                m[f"band{pos}"] = zero_band
                continue
            b, ylo, pxs = slots[k]
            band = np.zeros((BANDPX, C), dtype=np.float16)
            lo, hi = ylo * W, min((ylo + BROWS) * W, HW)
            band[:hi - lo] = ft[b][lo:hi]
            m[f"band{pos}"] = band
            n = pxs.size
            qq = np.zeros((cap, 2), dtype=np.int16)
            qq[:n] = (off[b, pxs] - lo).astype(np.int16)             # y0/y1 row starts
            w4 = np.zeros((cap, 4), dtype=np.float16)
            w4[:n] = wts[b, pxs]
            # idx pos within chunk = (2s+t)*128+p ; wrapped in 16 partitions
            arr = qq.reshape(cpb, 4, 128, 2).transpose(0, 1, 3, 2)   # j2,s,t,p
            arr = arr.reshape(cpb, 64, 16).transpose(2, 0, 1)        # part16,j2,col
            cols = slice(pos * cpb * 64, (pos + 1) * cpb * 64)
            for cc in range(8):
                m["idx"][16 * cc:16 * (cc + 1), cols] = arr.reshape(16, cpb * 64)
            wv = w4.reshape(cpb, 4, 128, 4).transpose(2, 0, 1, 3)    # p,j2,s,4
            m["wts"][:, pos * cpb * 16:(pos + 1) * cpb * 16] = \
                wv.reshape(128, cpb * 16)
            r0 = pos * cpb * 512
            rows_b[r0:r0 + n] = b
            rows_px[r0:r0 + n] = pxs
        in_maps2.append(m)
        meta.append((rows_b, rows_px))
    r2 = run_bass_kernel_spmd(nc2, in_maps2, core_ids=list(range(NCORES)), trace=TRACE)
    LAST_RESULTS["sample"] = r2

    res = np.zeros((B, HW, C), dtype=np.float32)
    for core in range(NCORES):
        rows_b, rows_px = meta[core]
        sel = rows_b >= 0
        res[rows_b[sel], rows_px[sel]] = r2.results[core]["out_t"][sel]
    out = np.ascontiguousarray(res.transpose(0, 2, 1)).reshape(B, C, H, W)
    for b in ident_b:
        out[b] = feats[b]
    return out



# revision 20
# speedup vs baseline: 1.1718x; 1.1718x over previous
"""Trainium2 Bass kernel for nn_Correction (nms_detection).

Strategy: data-parallel over batch (1 batch per NeuronCore, 8 cores).
  NEFF1 (device): desc = relu(conv3x3(feats, w_pa) + b_pa)  -- fp32 matmuls
                  (the precision-critical, FLOP-dominant stage)
  host:           scores = sigmoid(1x1conv(desc)); simple_nms; top-k;
                  gather+normalize kd; cross-batch attention (gnn); proj;
                  mind -> theta -> affine grid -> gather offsets + bilinear
                  weights  (all <0.1% of total FLOPs)
  NEFF2 (device): out = bilinear grid_sample of feats via indirect-DMA
                  row-pair gathers + per-partition weighted combine
"""

import functools
import numpy as np

import concourse.bacc as bacc
import concourse.bass as bass
import concourse.mybir as mybir
import concourse.tile as tile
from concourse.bass import IndirectOffsetOnAxis
from concourse.bass_utils import run_bass_kernel_spmd

B, C, H, W = 8, 256, 128, 384
CH = 128            # C // 2, desc channels
HW = H * W          # 49152
K = 1024            # MAX_KPTS
NMS_R = 4
NCORES = 8
F32 = mybir.dt.float32
I32 = mybir.dt.int32

# test.py can flip these to profile
TRACE = False
LAST_RESULTS = {}

# ----------------------------------------------------------------------------
# NEFF 1: fp32 3x3 conv + bias + relu.   feats [256,128,384] -> desc [128,128,384]
# ----------------------------------------------------------------------------

CONV_BF16 = True   # bf16 hi/lo split (3 passes) instead of fp32 (4 cyc/row)
BF16 = mybir.dt.bfloat16


@functools.lru_cache(maxsize=1)
def _build_conv():
    nc = bacc.Bacc("TRN2", target_bir_lowering=False, debug=False, num_devices=NCORES)
    if CONV_BF16:
        fh_d = nc.dram_tensor("feats_hi", [C, H, W], BF16, kind="ExternalInput")
        fl_d = nc.dram_tensor("feats_lo", [C, H, W], BF16, kind="ExternalInput")
        wh_d = nc.dram_tensor("w_hi", [128, 18 * 128], BF16, kind="ExternalInput")
        wl_d = nc.dram_tensor("w_lo", [128, 18 * 128], BF16, kind="ExternalInput")
        feat_aps = [fh_d.ap(), fl_d.ap()]
    else:
        feats_d = nc.dram_tensor("feats", [C, H, W], F32, kind="ExternalInput")
        w_d = nc.dram_tensor("w_all", [128, 18 * 128], F32, kind="ExternalInput")
        feat_aps = [feats_d.ap()]
    b_d = nc.dram_tensor("bias", [128, 1], F32, kind="ExternalInput")
    desc_d = nc.dram_tensor("desc", [CH, H, W], F32, kind="ExternalOutput")
    desc_ap = desc_d.ap()
    rdt = BF16 if CONV_BF16 else F32
    nparts = len(feat_aps)  # hi/lo parts of the input

    with tile.TileContext(nc) as tc:
        with (
            tc.tile_pool(name="const", bufs=1) as constp,
            tc.tile_pool(name="rows", bufs=10) as rowp,
            tc.tile_pool(name="out", bufs=4) as outp,
            tc.tile_pool(name="ps", bufs=2, space="PSUM") as psp,
        ):
            if CONV_BF16:
                w_hi = constp.tile([128, 18 * 128], BF16)
                nc.sync.dma_start(w_hi[:], wh_d.ap())
                w_lo = constp.tile([128, 18 * 128], BF16)
                nc.sync.dma_start(w_lo[:], wl_d.ap())
            else:
                w_all = constp.tile([128, 18 * 128], F32)
                nc.sync.dma_start(w_all[:], w_d.ap())
            bias_t = constp.tile([128, 1], F32)
            nc.sync.dma_start(bias_t[:], b_d.ap())
            zrow = [constp.tile([128, W + 2], rdt, tag=f"z{g}{v}", name=f"zrow{g}{v}")
                    for g in range(2) for v in range(nparts)]
            for z in zrow:
                nc.gpsimd.memset(z[:], 0.0)

            def load_row(h):
                # [(g0 hi, g0 lo), (g1 hi, g1 lo)] for image row h (zeros if OOB)
                if h < 0 or h >= H:
                    return [zrow[0:nparts], zrow[nparts:2 * nparts]]
                out = []
                for g in range(2):
                    tv = []
                    for v in range(nparts):
                        t = rowp.tile([128, W + 2], rdt, tag=f"row{g}{v}",
                                      name=f"row{g}{v}")
                        nc.gpsimd.memset(t[:, 0:1], 0.0)
                        nc.gpsimd.memset(t[:, W + 1:W + 2], 0.0)
                        nc.sync.dma_start(t[:, 1:W + 1],
                                          feat_aps[v][g * 128:(g + 1) * 128, h, :])
                        tv.append(t)
                    out.append(tv)
                return out

            window = {}  # h -> [[g0 parts], [g1 parts]]
            for h in range(H):
                for hh in (h - 1, h, h + 1):
                    if hh not in window:
                        window[hh] = load_row(hh)
                ps = psp.tile([128, W], F32)
                nmm = 18 * (3 if CONV_BF16 else 1)
                k = 0
                mm = 0
                for ky in range(3):
                    for kx in range(3):
                        rt = window[h + ky - 1]
                        for g in range(2):
                            ws = slice(k * 128, (k + 1) * 128)
                            if CONV_BF16:
                                # w_hi*x_hi + w_hi*x_lo + w_lo*x_hi
                                for wt, xv in ((w_hi, 0), (w_hi, 1), (w_lo, 0)):
                                    nc.tensor.matmul(
                                        ps[:], wt[:, ws], rt[g][xv][:, kx:kx + W],
                                        start=(mm == 0), stop=(mm == nmm - 1))
                                    mm += 1
                            else:
                                nc.tensor.matmul(
                                    ps[:], w_all[:, ws], rt[g][0][:, kx:kx + W],
                                    start=(mm == 0), stop=(mm == nmm - 1))
                                mm += 1
                            k += 1
                ot = outp.tile([128, W], F32)
                nc.scalar.activation(ot[:], ps[:], mybir.ActivationFunctionType.Relu,
                                     bias=bias_t[:, 0:1], scale=1.0)
                nc.sync.dma_start(desc_ap[:, h, :], ot[:])
                # drop the row leaving the window
                window.pop(h - 1, None)
    nc.compile()
    return nc


# ----------------------------------------------------------------------------
# NEFF 2: grid_sample over valid pixels only, redistributed across all cores.
#   Each core gets 4 DRAM "band" tensors (65 image rows of some batch, fp16).
#   Pixels are packed into superchunks of 512; chunk j gathers from band
#   j // (nchunk//4).  Per pixel: 2 elements of 2 px (1024B) on rows y0/y1,
#   4 weights.  prepare_only gathers + trigger_dma keep GPSIMD free.
# ----------------------------------------------------------------------------

NSUB = 4            # 4 x 128 pixels per superchunk
BROWS = 65          # rows per band tensor
BANDPX = BROWS * W  # 24960 pixel-rows per band
GE2 = 2 * C         # gather element: 2 pixel-rows (1024B fp16)
GSTEP2 = C          # element stride: 1 pixel-row -> idx = px index, fits int16
NQ2 = BANDPX - 1
FP16 = mybir.dt.float16
PREP_GATHER = False   # prepare_only + trigger_dma (frees GPSIMD) vs blocking


@functools.lru_cache(maxsize=4)
def _build_sample2(nchunk, prep=PREP_GATHER):
    nc = bacc.Bacc("TRN2", target_bir_lowering=False, debug=False, num_devices=NCORES)
    band_d = [nc.dram_tensor(f"band{v}", [BANDPX, C], FP16, kind="ExternalInput")
              for v in range(4)]
    idx_d = nc.dram_tensor("idx", [128, nchunk * 64], mybir.dt.int16,
                           kind="ExternalInput")
    wts_d = nc.dram_tensor("wts", [128, nchunk * 16], FP16, kind="ExternalInput")
    out_d = nc.dram_tensor("out_t", [nchunk * 512, C], F32, kind="ExternalOutput")
    band_views = [bass.AP(d.ap().tensor, 0, [[GSTEP2, NQ2], [1, GE2]])
                  for d in band_d]
    out_ap = out_d.ap()
    cpb = nchunk // 4   # chunks per band

    with tile.TileContext(nc) as tc:
        with (
            tc.tile_pool(name="const", bufs=1) as constp,
            tc.tile_pool(name="gat", bufs=6) as gatp,
            tc.tile_pool(name="prod", bufs=12) as prodp,
            tc.tile_pool(name="out", bufs=4) as outp,
            tc.tile_pool(name="ps", bufs=6, space="PSUM") as psp,
        ):
            idx_t = constp.tile([128, nchunk * 64], mybir.dt.int16)
            nc.sync.dma_start(idx_t[:], idx_d.ap())
            wts_t = constp.tile([128, nchunk * 16], FP16)
            nc.sync.dma_start(wts_t[:], wts_d.ap())
            ones_t = constp.tile([128, 128], FP16)
            nc.gpsimd.memset(ones_t[:], 1.0)
            ident = constp.tile([128, 128], FP16)
            nc.gpsimd.affine_select(ident[:], ones_t[:], pattern=[[1, 128]],
                                    compare_op=mybir.AluOpType.is_equal, fill=0.0,
                                    base=0, channel_multiplier=-1)
            for j in range(nchunk):
                g = gatp.tile([128, 2 * NSUB * GE2], FP16, tag="g")
                nc.gpsimd.dma_gather(
                    g[:].rearrange("p (i e) -> p i e", e=GE2),
                    band_views[j // cpb], idx_t[:, j * 64:(j + 1) * 64],
                    num_idxs=2 * NSUB * 128, num_idxs_reg=2 * NSUB * 128,
                    elem_size=GE2, elem_step=GSTEP2)
                ot = outp.tile([128, NSUB * C], F32, tag="ot")
                for s in range(NSUB):
                    ps = psp.tile([128, C], F32)
                    # one DVE mul for all 4 taps: [128, (4,256)] * w[128,(4,1->256)]
                    pr = prodp.tile([128, 4 * C], FP16, tag="pr")
                    src = g[:, 2 * s * GE2:2 * s * GE2 + 4 * C]
                    w4 = wts_t[:, j * 16 + s * 4:j * 16 + s * 4 + 4]
                    w_b = bass.AP(w4.tensor, w4.offset, list(w4.ap) + [[0, C]])
                    nc.vector.tensor_tensor(
                        pr[:].rearrange("p (k c) -> p k c", c=C),
                        src.rearrange("p (k c) -> p k c", c=C),
                        w_b, mybir.AluOpType.mult)
                    for k2 in range(4):
                        nc.tensor.matmul(ps[:], ident[:],
                                         pr[:, k2 * C:(k2 + 1) * C],
                                         start=(k2 == 0), stop=(k2 == 3))
                    nc.scalar.activation(ot[:, s * C:(s + 1) * C], ps[:],
                                         mybir.ActivationFunctionType.Copy, scale=1.0)
                dst = out_ap[j * 512:(j + 1) * 512, :].rearrange("(s p) c -> p s c", p=128)
                nc.sync.dma_start(dst, ot[:].rearrange("p (s c) -> p s c", s=NSUB))
    nc.compile()
    return nc


# ----------------------------------------------------------------------------
# Host-side middle stages (tiny compute)
# ----------------------------------------------------------------------------

def _max_pool(x, r):
    b, h, w = x.shape
    k = 2 * r + 1
    xp = np.pad(x, ((0, 0), (r, r), (r, r)), constant_values=-np.inf)
    out = np.full((b, h, w), -np.inf, dtype=x.dtype)
    for dy in range(k):
        for dx in range(k):
            np.maximum(out, xp[:, dy:dy + h, dx:dx + w], out=out)
    return out


def _simple_nms(scores, r):
    zeros = np.zeros_like(scores)
    max_mask = scores == _max_pool(scores, r)
    for _ in range(2):
        supp_mask = _max_pool(max_mask.astype(scores.dtype), r) > 0
        supp_scores = np.where(supp_mask, zeros, scores)
        new_max_mask = supp_scores == _max_pool(supp_scores, r)
        max_mask = max_mask | (new_max_mask & ~supp_mask)
    return np.where(max_mask, scores, zeros)


def _host_middle(desc, feats, w_pb, b_pb, w_proj, b_proj):
    """desc [B, CH, HW] f32 (device conv output) -> theta [B, 2, 3] f32."""
    pre = np.einsum('bcp,c->bp', desc, w_pb[:, :, 0, 0][0], optimize=True) + b_pb[0]
    scores = 1.0 / (1.0 + np.exp(-pre.astype(np.float32)))
    scores = _simple_nms(scores.reshape(B, H, W), NMS_R).reshape(B, -1)
    idx = np.argsort(-scores, axis=1, kind='stable')[:, :K]          # [B, K]
    kd = np.take_along_axis(desc, idx[:, None, :], axis=2)           # [B, CH, K]
    norm = np.sqrt(np.sum(kd * kd, axis=1, keepdims=True))
    kd = kd / np.maximum(norm, 1e-12)
    # gnn: per-keypoint attention across the batch dim
    q = np.transpose(kd, (2, 0, 1)).astype(np.float32)               # [K, B, CH]
    sc = np.einsum('lnc,lmc->lnm', q, q, optimize=True) / np.float32(np.sqrt(CH))
    sc = sc - sc.max(axis=-1, keepdims=True)
    e = np.exp(sc)
    prob = e / e.sum(-1, keepdims=True)
    msg = np.einsum('lnm,lmc->lnc', prob, q, optimize=True)
    kd2 = kd + (kd + np.transpose(msg, (1, 2, 0)))
    proj = np.einsum('bcl,oc->bol', kd2, w_proj[:, :, 0], optimize=True) \
        + b_proj[None, :, None]
    proj = proj - proj[0:1]
    mind = proj.min(axis=2).astype(np.float32)                       # [B, 3]
    c, s = np.cos(mind[:, 2]), np.sin(mind[:, 2])
    theta = np.stack([np.stack([c, -s, mind[:, 0]], -1),
                      np.stack([s, c, mind[:, 1]], -1)], axis=1).astype(np.float32)
    return theta


def _grid_tables(theta):
    """theta [B,2,3] -> off [B, HW, 2] int32 row starts, wts [B, HW, 4] f32."""
    xs = ((np.arange(W, dtype=np.float32) * 2 + 1) / W - 1)
    ys = ((np.arange(H, dtype=np.float32) * 2 + 1) / H - 1)
    gxm, gym = np.meshgrid(xs, ys)                                   # [H, W]
    offs, wtss = [], []
    for b in range(B):
        t = theta[b]
        grid_x = gxm * t[0, 0] + gym * t[0, 1] + t[0, 2]
        grid_y = gxm * t[1, 0] + gym * t[1, 1] + t[1, 2]
        gx = (grid_x + 1) * W / 2 - 0.5
        gy = (grid_y + 1) * H / 2 - 0.5
        x0 = np.floor(gx)
        y0 = np.floor(gy)
        wx1 = (gx - x0).astype(np.float32); wx0 = 1.0 - wx1
        wy1 = (gy - y0).astype(np.float32); wy0 = 1.0 - wy1

        def v(xi, yi):
            return ((xi >= 0) & (xi < W) & (yi >= 0) & (yi < H)).astype(np.float32)
        w00 = wx0 * wy0 * v(x0, y0)
        w01 = wx1 * wy0 * v(x0 + 1, y0)
        w10 = wx0 * wy1 * v(x0, y0 + 1)
        w11 = wx1 * wy1 * v(x0 + 1, y0 + 1)
        x0i = x0.astype(np.int64)
        xs_ = np.clip(x0i, 0, W - 2)
        wa0 = w00 * (xs_ == x0i) + w01 * (xs_ == x0i + 1)
        wb0 = w00 * (xs_ + 1 == x0i) + w01 * (xs_ + 1 == x0i + 1)
        wa1 = w10 * (xs_ == x0i) + w11 * (xs_ == x0i + 1)
        wb1 = w10 * (xs_ + 1 == x0i) + w11 * (xs_ + 1 == x0i + 1)
        y0i = y0.astype(np.int64)
        y0c = np.clip(y0i, 0, H - 1)
        y1c = np.clip(y0i + 1, 0, H - 1)
        off0 = (y0c * W + xs_).astype(np.int32)
        off1 = (y1c * W + xs_).astype(np.int32)
        offs.append(np.stack([off0.reshape(-1), off1.reshape(-1)], -1))
        wtss.append(np.stack([wa0.reshape(-1), wb0.reshape(-1),
                              wa1.reshape(-1), wb1.reshape(-1)], -1).astype(np.float32))
    return np.stack(offs), np.stack(wtss)


# ----------------------------------------------------------------------------
# kernel()
# ----------------------------------------------------------------------------

def kernel(feats, w_pa, b_pa, w_pb, b_pb, w_proj, b_proj):
    import ml_dtypes
    feats = np.ascontiguousarray(feats, dtype=np.float32)
    # weights for the conv matmuls: block k=((ky*3+kx)*2+g): lhsT[ci, co]
    wr = w_pa.reshape(128, 2, 128, 3, 3).transpose(2, 3, 4, 1, 0)   # ci,ky,kx,g,co
    w_all = np.ascontiguousarray(wr.reshape(128, 18 * 128), dtype=np.float32)
    bias = np.ascontiguousarray(b_pa.reshape(128, 1), dtype=np.float32)

    nc1 = _build_conv()
    if CONV_BF16:
        f_hi = feats.astype(ml_dtypes.bfloat16)
        f_lo = (feats - f_hi.astype(np.float32)).astype(ml_dtypes.bfloat16)
        w_hi = w_all.astype(ml_dtypes.bfloat16)
        w_lo = (w_all - w_hi.astype(np.float32)).astype(ml_dtypes.bfloat16)
        in_maps = [{"feats_hi": f_hi[b], "feats_lo": f_lo[b],
                    "w_hi": w_hi, "w_lo": w_lo, "bias": bias} for b in range(B)]
    else:
        in_maps = [{"feats": feats[b], "w_all": w_all, "bias": bias} for b in range(B)]
    r1 = run_bass_kernel_spmd(nc1, in_maps, core_ids=list(range(NCORES)), trace=TRACE)
    LAST_RESULTS["conv"] = r1
    desc = np.stack([r1.results[b]["desc"] for b in range(B)])       # [B, CH, H, W]

    theta = _host_middle(desc.reshape(B, CH, HW), feats, w_pb, b_pb, w_proj, b_proj)
    off, wts = _grid_tables(theta)                                   # [B,HW,2],[B,HW,4]

    # batches whose theta is exactly identity sample the image exactly -> copy
    ident_b = [b for b in range(B)
               if np.abs(theta[b] - np.array([[1, 0, 0], [0, 1, 0]],
                                             dtype=np.float32)).max() < 1e-6]
    work_b = [b for b in range(B) if b not in ident_b]

    # valid pixels: at least one of the 4 bilinear weights nonzero
    wsum = wts.sum(axis=2)                                           # [B, HW]
    slots = None
    for nchunk in (48, 56, 64, 80, 96, 112, 128, 160, 192):
        cap = (nchunk // 4) * 512
        cand = []
        feasible = True
        for b in work_b:
            pxs = np.nonzero(wsum[b] > 0)[0].astype(np.int64)
            if pxs.size == 0:
                continue
            y0c = off[b, pxs, 0] // W                                # source row of y0 tap
            order = np.argsort(y0c, kind="stable")
            pxs, y0c = pxs[order], y0c[order]
            i = 0
            while i < pxs.size:
                ylo = int(y0c[i])
                j = min(i + cap, int(np.searchsorted(y0c, ylo + 64, "left")))
                cand.append((b, ylo, pxs[i:j]))
                i = j
            if len(cand) > 32:
                feasible = False
                break
        if feasible and len(cand) <= 32:
            slots = cand
            break
    assert slots is not None, "could not pack valid pixels into 32 slots"

    nc2 = _build_sample2(nchunk, PREP_GATHER)
    cpb = nchunk // 4
    ft = {b: feats[b].reshape(C, HW).T.astype(np.float16) for b in work_b}

    zero_band = np.zeros((BANDPX, C), dtype=np.float16)
    in_maps2 = []
    meta = []                                                        # per core: (b, px) rows
    for core in range(NCORES):
        m = {"idx": np.zeros((128, nchunk * 64), dtype=np.int16),
             "wts": np.zeros((128, nchunk * 16), dtype=np.float16)}
        rows_b = np.full(nchunk * 512, -1, dtype=np.int64)
        rows_px = np.zeros(nchunk * 512, dtype=np.int64)
        for pos in range(4):
            k = core * 4 + pos
            if k >= len(slots):
                m[f"band{pos}"] = zero_band
                continue
            b, ylo, pxs = slots[k]
            band = np.zeros((BANDPX, C), dtype=np.float16)
            lo, hi = ylo * W, min((ylo + BROWS) * W, HW)
            band[:hi - lo] = ft[b][lo:hi]
            m[f"band{pos}"] = band
            n = pxs.size
            qq = np.zeros((cap, 2), dtype=np.int16)
            qq[:n] = (off[b, pxs] - lo).astype(np.int16)             # y0/y1 row starts
            w4 = np.zeros((cap, 4), dtype=np.float16)
            w4[:n] = wts[b, pxs]
            # idx pos within chunk = (2s+t)*128+p ; wrapped in 16 partitions
            arr = qq.reshape(cpb, 4, 128, 2).transpose(0, 1, 3, 2)   # j2,s,t,p
            arr = arr.reshape(cpb, 64, 16).transpose(2, 0, 1)        # part16,j2,col
            cols = slice(pos * cpb * 64, (pos + 1) * cpb * 64)
            for cc in range(8):
                m["idx"][16 * cc:16 * (cc + 1), cols] = arr.reshape(16, cpb * 64)
            wv = w4.reshape(cpb, 4, 128, 4).transpose(2, 0, 1, 3)    # p,j2,s,4
            m["wts"][:, pos * cpb * 16:(pos + 1) * cpb * 16] = \
                wv.reshape(128, cpb * 16)
            r0 = pos * cpb * 512
            rows_b[r0:r0 + n] = b
            rows_px[r0:r0 + n] = pxs
        in_maps2.append(m)
        meta.append((rows_b, rows_px))
    r2 = run_bass_kernel_spmd(nc2, in_maps2, core_ids=list(range(NCORES)), trace=TRACE)
    LAST_RESULTS["sample"] = r2

    res = np.zeros((B, HW, C), dtype=np.float32)
    for core in range(NCORES):
        rows_b, rows_px = meta[core]
        sel = rows_b >= 0
        res[rows_b[sel], rows_px[sel]] = r2.results[core]["out_t"][sel]
    out = np.ascontiguousarray(res.transpose(0, 2, 1)).reshape(B, C, H, W)
    for b in ident_b:
        out[b] = feats[b]
    return out



# revision 21
# speedup vs baseline: 1.2117x; 1.0341x over previous
"""Trainium2 Bass kernel for nn_Correction (nms_detection).

Strategy: data-parallel over batch (1 batch per NeuronCore, 8 cores).
  NEFF1 (device): desc = relu(conv3x3(feats, w_pa) + b_pa)  -- fp32 matmuls
                  (the precision-critical, FLOP-dominant stage)
  host:           scores = sigmoid(1x1conv(desc)); simple_nms; top-k;
                  gather+normalize kd; cross-batch attention (gnn); proj;
                  mind -> theta -> affine grid -> gather offsets + bilinear
                  weights  (all <0.1% of total FLOPs)
  NEFF2 (device): out = bilinear grid_sample of feats via indirect-DMA
                  row-pair gathers + per-partition weighted combine
"""

import functools
import numpy as np

import concourse.bacc as bacc
import concourse.bass as bass
import concourse.mybir as mybir
import concourse.tile as tile
from concourse.bass import IndirectOffsetOnAxis
from concourse.bass_utils import run_bass_kernel_spmd

B, C, H, W = 8, 256, 128, 384
CH = 128            # C // 2, desc channels
HW = H * W          # 49152
K = 1024            # MAX_KPTS
NMS_R = 4
NCORES = 8
F32 = mybir.dt.float32
I32 = mybir.dt.int32

# test.py can flip these to profile
TRACE = False
LAST_RESULTS = {}

# ----------------------------------------------------------------------------
# NEFF 1: fp32 3x3 conv + bias + relu.   feats [256,128,384] -> desc [128,128,384]
# ----------------------------------------------------------------------------

CONV_BF16 = True   # bf16 hi/lo split (3 passes) instead of fp32 (4 cyc/row)
BF16 = mybir.dt.bfloat16


@functools.lru_cache(maxsize=1)
def _build_conv():
    nc = bacc.Bacc("TRN2", target_bir_lowering=False, debug=False, num_devices=NCORES)
    if CONV_BF16:
        fh_d = nc.dram_tensor("feats_hi", [C, H, W], BF16, kind="ExternalInput")
        fl_d = nc.dram_tensor("feats_lo", [C, H, W], BF16, kind="ExternalInput")
        wh_d = nc.dram_tensor("w_hi", [128, 18 * 128], BF16, kind="ExternalInput")
        wl_d = nc.dram_tensor("w_lo", [128, 18 * 128], BF16, kind="ExternalInput")
        feat_aps = [fh_d.ap(), fl_d.ap()]
    else:
        feats_d = nc.dram_tensor("feats", [C, H, W], F32, kind="ExternalInput")
        w_d = nc.dram_tensor("w_all", [128, 18 * 128], F32, kind="ExternalInput")
        feat_aps = [feats_d.ap()]
    b_d = nc.dram_tensor("bias", [128, 1], F32, kind="ExternalInput")
    desc_d = nc.dram_tensor("desc", [CH, H, W], F32, kind="ExternalOutput")
    desc_ap = desc_d.ap()
    rdt = BF16 if CONV_BF16 else F32
    nparts = len(feat_aps)  # hi/lo parts of the input

    with tile.TileContext(nc) as tc:
        with (
            tc.tile_pool(name="const", bufs=1) as constp,
            tc.tile_pool(name="rows", bufs=10) as rowp,
            tc.tile_pool(name="out", bufs=4) as outp,
            tc.tile_pool(name="ps", bufs=2, space="PSUM") as psp,
        ):
            if CONV_BF16:
                w_hi = constp.tile([128, 18 * 128], BF16)
                nc.sync.dma_start(w_hi[:], wh_d.ap())
                w_lo = constp.tile([128, 18 * 128], BF16)
                nc.sync.dma_start(w_lo[:], wl_d.ap())
            else:
                w_all = constp.tile([128, 18 * 128], F32)
                nc.sync.dma_start(w_all[:], w_d.ap())
            bias_t = constp.tile([128, 1], F32)
            nc.sync.dma_start(bias_t[:], b_d.ap())
            zrow = [constp.tile([128, W + 2], rdt, tag=f"z{g}{v}", name=f"zrow{g}{v}")
                    for g in range(2) for v in range(nparts)]
            for z in zrow:
                nc.gpsimd.memset(z[:], 0.0)

            def load_row(h):
                # [(g0 hi, g0 lo), (g1 hi, g1 lo)] for image row h (zeros if OOB)
                if h < 0 or h >= H:
                    return [zrow[0:nparts], zrow[nparts:2 * nparts]]
                out = []
                for g in range(2):
                    tv = []
                    for v in range(nparts):
                        t = rowp.tile([128, W + 2], rdt, tag=f"row{g}{v}",
                                      name=f"row{g}{v}")
                        nc.gpsimd.memset(t[:, 0:1], 0.0)
                        nc.gpsimd.memset(t[:, W + 1:W + 2], 0.0)
                        nc.sync.dma_start(t[:, 1:W + 1],
                                          feat_aps[v][g * 128:(g + 1) * 128, h, :])
                        tv.append(t)
                    out.append(tv)
                return out

            window = {}  # h -> [[g0 parts], [g1 parts]]
            for h in range(H):
                for hh in (h - 1, h, h + 1):
                    if hh not in window:
                        window[hh] = load_row(hh)
                ps = psp.tile([128, W], F32)
                nmm = 18 * (3 if CONV_BF16 else 1)
                k = 0
                mm = 0
                for ky in range(3):
                    for kx in range(3):
                        rt = window[h + ky - 1]
                        for g in range(2):
                            ws = slice(k * 128, (k + 1) * 128)
                            if CONV_BF16:
                                # w_hi*x_hi + w_hi*x_lo + w_lo*x_hi
                                for wt, xv in ((w_hi, 0), (w_hi, 1), (w_lo, 0)):
                                    nc.tensor.matmul(
                                        ps[:], wt[:, ws], rt[g][xv][:, kx:kx + W],
                                        start=(mm == 0), stop=(mm == nmm - 1))
                                    mm += 1
                            else:
                                nc.tensor.matmul(
                                    ps[:], w_all[:, ws], rt[g][0][:, kx:kx + W],
                                    start=(mm == 0), stop=(mm == nmm - 1))
                                mm += 1
                            k += 1
                ot = outp.tile([128, W], F32)
                nc.scalar.activation(ot[:], ps[:], mybir.ActivationFunctionType.Relu,
                                     bias=bias_t[:, 0:1], scale=1.0)
                nc.sync.dma_start(desc_ap[:, h, :], ot[:])
                # drop the row leaving the window
                window.pop(h - 1, None)
    nc.compile()
    return nc


# ----------------------------------------------------------------------------
# NEFF 2: grid_sample over valid pixels only, redistributed across all cores.
#   Each core gets 4 DRAM "band" tensors (65 image rows of some batch, fp16).
#   Pixels are packed into superchunks of 512; chunk j gathers from band
#   j // (nchunk//4).  Per pixel: 2 elements of 2 px (1024B) on rows y0/y1,
#   4 weights.  prepare_only gathers + trigger_dma keep GPSIMD free.
# ----------------------------------------------------------------------------

NSUB = 4            # 4 x 128 pixels per superchunk
BROWS = 65          # rows per band tensor
BANDPX = BROWS * W  # 24960 pixel-rows per band
GE2 = 2 * C         # gather element: 2 pixel-rows (1024B fp16)
GSTEP2 = C          # element stride: 1 pixel-row -> idx = px index, fits int16
NQ2 = BANDPX - 1
FP16 = mybir.dt.float16
PREP_GATHER = False   # prepare_only + trigger_dma (frees GPSIMD) vs blocking


@functools.lru_cache(maxsize=4)
def _build_sample2(nchunk, prep=PREP_GATHER):
    nc = bacc.Bacc("TRN2", target_bir_lowering=False, debug=False, num_devices=NCORES)
    band_d = [nc.dram_tensor(f"band{v}", [BANDPX, C], FP16, kind="ExternalInput")
              for v in range(4)]
    idx_d = nc.dram_tensor("idx", [128, nchunk * 64], mybir.dt.int16,
                           kind="ExternalInput")
    wts_d = nc.dram_tensor("wts", [128, nchunk * 16], FP16, kind="ExternalInput")
    out_d = nc.dram_tensor("out_t", [nchunk * 512, C], F32, kind="ExternalOutput")
    band_views = [bass.AP(d.ap().tensor, 0, [[GSTEP2, NQ2], [1, GE2]])
                  for d in band_d]
    out_ap = out_d.ap()
    cpb = nchunk // 4   # chunks per band

    with tile.TileContext(nc) as tc:
        with (
            tc.tile_pool(name="const", bufs=1) as constp,
            tc.tile_pool(name="gat", bufs=6) as gatp,
            tc.tile_pool(name="prod", bufs=12) as prodp,
            tc.tile_pool(name="out", bufs=4) as outp,
            tc.tile_pool(name="ps", bufs=6, space="PSUM") as psp,
        ):
            idx_t = constp.tile([128, nchunk * 64], mybir.dt.int16)
            nc.sync.dma_start(idx_t[:], idx_d.ap())
            wts_t = constp.tile([128, nchunk * 16], FP16)
            nc.sync.dma_start(wts_t[:], wts_d.ap())
            ones_t = constp.tile([128, 128], FP16)
            nc.gpsimd.memset(ones_t[:], 1.0)
            ident = constp.tile([128, 128], FP16)
            nc.gpsimd.affine_select(ident[:], ones_t[:], pattern=[[1, 128]],
                                    compare_op=mybir.AluOpType.is_equal, fill=0.0,
                                    base=0, channel_multiplier=-1)
            for j in range(nchunk):
                g = gatp.tile([128, 2 * NSUB * GE2], FP16, tag="g")
                nc.gpsimd.dma_gather(
                    g[:].rearrange("p (i e) -> p i e", e=GE2),
                    band_views[j // cpb], idx_t[:, j * 64:(j + 1) * 64],
                    num_idxs=2 * NSUB * 128, num_idxs_reg=2 * NSUB * 128,
                    elem_size=GE2, elem_step=GSTEP2)
                ot = outp.tile([128, NSUB * C], F32, tag="ot")
                for s in range(NSUB):
                    ps = psp.tile([128, C], F32)
                    # one DVE mul for all 4 taps: [128, (4,256)] * w[128,(4,1->256)]
                    pr = prodp.tile([128, 4 * C], FP16, tag="pr")
                    src = g[:, 2 * s * GE2:2 * s * GE2 + 4 * C]
                    w4 = wts_t[:, j * 16 + s * 4:j * 16 + s * 4 + 4]
                    w_b = bass.AP(w4.tensor, w4.offset, list(w4.ap) + [[0, C]])
                    nc.vector.tensor_tensor(
                        pr[:].rearrange("p (k c) -> p k c", c=C),
                        src.rearrange("p (k c) -> p k c", c=C),
                        w_b, mybir.AluOpType.mult)
                    for k2 in range(4):
                        nc.tensor.matmul(ps[:], ident[:],
                                         pr[:, k2 * C:(k2 + 1) * C],
                                         start=(k2 == 0), stop=(k2 == 3))
                    nc.scalar.activation(ot[:, s * C:(s + 1) * C], ps[:],
                                         mybir.ActivationFunctionType.Copy, scale=1.0)
                dst = out_ap[j * 512:(j + 1) * 512, :].rearrange("(s p) c -> p s c", p=128)
                nc.sync.dma_start(dst, ot[:].rearrange("p (s c) -> p s c", s=NSUB))
    nc.compile()
    return nc


# ----------------------------------------------------------------------------
# Host-side middle stages (tiny compute)
# ----------------------------------------------------------------------------

def _max_pool(x, r):
    b, h, w = x.shape
    k = 2 * r + 1
    xp = np.pad(x, ((0, 0), (r, r), (r, r)), constant_values=-np.inf)
    out = np.full((b, h, w), -np.inf, dtype=x.dtype)
    for dy in range(k):
        for dx in range(k):
            np.maximum(out, xp[:, dy:dy + h, dx:dx + w], out=out)
    return out


def _simple_nms(scores, r):
    zeros = np.zeros_like(scores)
    max_mask = scores == _max_pool(scores, r)
    for _ in range(2):
        supp_mask = _max_pool(max_mask.astype(scores.dtype), r) > 0
        supp_scores = np.where(supp_mask, zeros, scores)
        new_max_mask = supp_scores == _max_pool(supp_scores, r)
        max_mask = max_mask | (new_max_mask & ~supp_mask)
    return np.where(max_mask, scores, zeros)


def _host_middle(desc, feats, w_pb, b_pb, w_proj, b_proj):
    """desc [B, CH, HW] f32 (device conv output) -> theta [B, 2, 3] f32."""
    pre = np.einsum('bcp,c->bp', desc, w_pb[:, :, 0, 0][0], optimize=True) + b_pb[0]
    scores = 1.0 / (1.0 + np.exp(-pre.astype(np.float32)))
    scores = _simple_nms(scores.reshape(B, H, W), NMS_R).reshape(B, -1)
    idx = np.argsort(-scores, axis=1, kind='stable')[:, :K]          # [B, K]
    kd = np.take_along_axis(desc, idx[:, None, :], axis=2)           # [B, CH, K]
    norm = np.sqrt(np.sum(kd * kd, axis=1, keepdims=True))
    kd = kd / np.maximum(norm, 1e-12)
    # gnn: per-keypoint attention across the batch dim
    q = np.transpose(kd, (2, 0, 1)).astype(np.float32)               # [K, B, CH]
    sc = np.einsum('lnc,lmc->lnm', q, q, optimize=True) / np.float32(np.sqrt(CH))
    sc = sc - sc.max(axis=-1, keepdims=True)
    e = np.exp(sc)
    prob = e / e.sum(-1, keepdims=True)
    msg = np.einsum('lnm,lmc->lnc', prob, q, optimize=True)
    kd2 = kd + (kd + np.transpose(msg, (1, 2, 0)))
    proj = np.einsum('bcl,oc->bol', kd2, w_proj[:, :, 0], optimize=True) \
        + b_proj[None, :, None]
    proj = proj - proj[0:1]
    mind = proj.min(axis=2).astype(np.float32)                       # [B, 3]
    c, s = np.cos(mind[:, 2]), np.sin(mind[:, 2])
    theta = np.stack([np.stack([c, -s, mind[:, 0]], -1),
                      np.stack([s, c, mind[:, 1]], -1)], axis=1).astype(np.float32)
    return theta


def _grid_tables(theta):
    """theta [B,2,3] -> off [B, HW, 2] int32 row starts, wts [B, HW, 4] f32."""
    xs = ((np.arange(W, dtype=np.float32) * 2 + 1) / W - 1)
    ys = ((np.arange(H, dtype=np.float32) * 2 + 1) / H - 1)
    gxm, gym = np.meshgrid(xs, ys)                                   # [H, W]
    offs, wtss = [], []
    for b in range(B):
        t = theta[b]
        grid_x = gxm * t[0, 0] + gym * t[0, 1] + t[0, 2]
        grid_y = gxm * t[1, 0] + gym * t[1, 1] + t[1, 2]
        gx = (grid_x + 1) * W / 2 - 0.5
        gy = (grid_y + 1) * H / 2 - 0.5
        x0 = np.floor(gx)
        y0 = np.floor(gy)
        wx1 = (gx - x0).astype(np.float32); wx0 = 1.0 - wx1
        wy1 = (gy - y0).astype(np.float32); wy0 = 1.0 - wy1

        def v(xi, yi):
            return ((xi >= 0) & (xi < W) & (yi >= 0) & (yi < H)).astype(np.float32)
        w00 = wx0 * wy0 * v(x0, y0)
        w01 = wx1 * wy0 * v(x0 + 1, y0)
        w10 = wx0 * wy1 * v(x0, y0 + 1)
        w11 = wx1 * wy1 * v(x0 + 1, y0 + 1)
        x0i = x0.astype(np.int64)
        xs_ = np.clip(x0i, 0, W - 2)
        wa0 = w00 * (xs_ == x0i) + w01 * (xs_ == x0i + 1)
        wb0 = w00 * (xs_ + 1 == x0i) + w01 * (xs_ + 1 == x0i + 1)
        wa1 = w10 * (xs_ == x0i) + w11 * (xs_ == x0i + 1)
        wb1 = w10 * (xs_ + 1 == x0i) + w11 * (xs_ + 1 == x0i + 1)
        y0i = y0.astype(np.int64)
        y0c = np.clip(y0i, 0, H - 1)
        y1c = np.clip(y0i + 1, 0, H - 1)
        off0 = (y0c * W + xs_).astype(np.int32)
        off1 = (y1c * W + xs_).astype(np.int32)
        offs.append(np.stack([off0.reshape(-1), off1.reshape(-1)], -1))
        wtss.append(np.stack([wa0.reshape(-1), wb0.reshape(-1),
                              wa1.reshape(-1), wb1.reshape(-1)], -1).astype(np.float32))
    return np.stack(offs), np.stack(wtss)


# ----------------------------------------------------------------------------
# kernel()
# ----------------------------------------------------------------------------

def kernel(feats, w_pa, b_pa, w_pb, b_pb, w_proj, b_proj):
    import ml_dtypes
    feats = np.ascontiguousarray(feats, dtype=np.float32)
    # weights for the conv matmuls: block k=((ky*3+kx)*2+g): lhsT[ci, co]
    wr = w_pa.reshape(128, 2, 128, 3, 3).transpose(2, 3, 4, 1, 0)   # ci,ky,kx,g,co
    w_all = np.ascontiguousarray(wr.reshape(128, 18 * 128), dtype=np.float32)
    bias = np.ascontiguousarray(b_pa.reshape(128, 1), dtype=np.float32)

    nc1 = _build_conv()
    if CONV_BF16:
        f_hi = feats.astype(ml_dtypes.bfloat16)
        f_lo = (feats - f_hi.astype(np.float32)).astype(ml_dtypes.bfloat16)
        w_hi = w_all.astype(ml_dtypes.bfloat16)
        w_lo = (w_all - w_hi.astype(np.float32)).astype(ml_dtypes.bfloat16)
        in_maps = [{"feats_hi": f_hi[b], "feats_lo": f_lo[b],
                    "w_hi": w_hi, "w_lo": w_lo, "bias": bias} for b in range(B)]
    else:
        in_maps = [{"feats": feats[b], "w_all": w_all, "bias": bias} for b in range(B)]
    r1 = run_bass_kernel_spmd(nc1, in_maps, core_ids=list(range(NCORES)), trace=TRACE)
    LAST_RESULTS["conv"] = r1
    desc = np.stack([r1.results[b]["desc"] for b in range(B)])       # [B, CH, H, W]

    theta = _host_middle(desc.reshape(B, CH, HW), feats, w_pb, b_pb, w_proj, b_proj)
    off, wts = _grid_tables(theta)                                   # [B,HW,2],[B,HW,4]

    # batches whose theta is exactly identity sample the image exactly -> copy
    ident_b = [b for b in range(B)
               if np.abs(theta[b] - np.array([[1, 0, 0], [0, 1, 0]],
                                             dtype=np.float32)).max() < 1e-6]
    work_b = [b for b in range(B) if b not in ident_b]

    # valid pixels: at least one of the 4 bilinear weights nonzero
    wsum = wts.sum(axis=2)                                           # [B, HW]
    slots = None
    for nchunk in (40, 44, 48, 56, 64, 80, 96, 112, 128, 160, 192):
        cap = (nchunk // 4) * 512
        cand = []
        feasible = True
        for b in work_b:
            pxs = np.nonzero(wsum[b] > 0)[0].astype(np.int64)
            if pxs.size == 0:
                continue
            y0c = off[b, pxs, 0] // W                                # source row of y0 tap
            order = np.argsort(y0c, kind="stable")
            pxs, y0c = pxs[order], y0c[order]
            i = 0
            while i < pxs.size:
                ylo = int(y0c[i])
                j = min(i + cap, int(np.searchsorted(y0c, ylo + 64, "left")))
                cand.append((b, ylo, pxs[i:j]))
                i = j
            if len(cand) > 32:
                feasible = False
                break
        if feasible and len(cand) <= 32:
            slots = cand
            break
    assert slots is not None, "could not pack valid pixels into 32 slots"
    print(f"sample packing: nchunk={nchunk} slots={len(slots)}")

    nc2 = _build_sample2(nchunk, PREP_GATHER)
    cpb = nchunk // 4
    ft = {b: feats[b].reshape(C, HW).T.astype(np.float16) for b in work_b}

    zero_band = np.zeros((BANDPX, C), dtype=np.float16)
    in_maps2 = []
    meta = []                                                        # per core: (b, px) rows
    for core in range(NCORES):
        m = {"idx": np.zeros((128, nchunk * 64), dtype=np.int16),
             "wts": np.zeros((128, nchunk * 16), dtype=np.float16)}
        rows_b = np.full(nchunk * 512, -1, dtype=np.int64)
        rows_px = np.zeros(nchunk * 512, dtype=np.int64)
        for pos in range(4):
            k = core * 4 + pos
            if k >= len(slots):
                m[f"band{pos}"] = zero_band
                continue
            b, ylo, pxs = slots[k]
            band = np.zeros((BANDPX, C), dtype=np.float16)
            lo, hi = ylo * W, min((ylo + BROWS) * W, HW)
            band[:hi - lo] = ft[b][lo:hi]
            m[f"band{pos}"] = band
            n = pxs.size
            qq = np.zeros((cap, 2), dtype=np.int16)
            qq[:n] = (off[b, pxs] - lo).astype(np.int16)             # y0/y1 row starts
            w4 = np.zeros((cap, 4), dtype=np.float16)
            w4[:n] = wts[b, pxs]
            # idx pos within chunk = (2s+t)*128+p ; wrapped in 16 partitions
            arr = qq.reshape(cpb, 4, 128, 2).transpose(0, 1, 3, 2)   # j2,s,t,p
            arr = arr.reshape(cpb, 64, 16).transpose(2, 0, 1)        # part16,j2,col
            cols = slice(pos * cpb * 64, (pos + 1) * cpb * 64)
            for cc in range(8):
                m["idx"][16 * cc:16 * (cc + 1), cols] = arr.reshape(16, cpb * 64)
            wv = w4.reshape(cpb, 4, 128, 4).transpose(2, 0, 1, 3)    # p,j2,s,4
            m["wts"][:, pos * cpb * 16:(pos + 1) * cpb * 16] = \
                wv.reshape(128, cpb * 16)
            r0 = pos * cpb * 512
            rows_b[r0:r0 + n] = b
            rows_px[r0:r0 + n] = pxs
        in_maps2.append(m)
        meta.append((rows_b, rows_px))
    r2 = run_bass_kernel_spmd(nc2, in_maps2, core_ids=list(range(NCORES)), trace=TRACE)
    LAST_RESULTS["sample"] = r2

    res = np.zeros((B, HW, C), dtype=np.float32)
    for core in range(NCORES):
        rows_b, rows_px = meta[core]
        sel = rows_b >= 0
        res[rows_b[sel], rows_px[sel]] = r2.results[core]["out_t"][sel]
    out = np.ascontiguousarray(res.transpose(0, 2, 1)).reshape(B, C, H, W)
    for b in ident_b:
        out[b] = feats[b]
    return out

